# revision 10
# baseline (speedup 1.0000x reference)
"""Trainium2 Bass kernel: multi-head self-attention with RoPE (causal).

Sharding: 8 cores = 2 batches x 4 head-groups. Core c handles batch c//4
and heads [4*(c%4), 4*(c%4)+4). Each core computes Q/K/V projections for
its 4 heads, RoPE, causal flash-style attention, and a partial output
projection (its heads' contribution to out @ Wo.T). Host sums the 4
partials per batch.

Layout notes:
- Weights are fed pre-transposed ([in, out]) so projections run with the
  contraction dim on partitions.
- Wq/Wk columns are permuted per head (even dims then odd dims) so RoPE
  becomes two 32-wide column-block ops instead of stride-2 ops. Scores
  are invariant to a shared permutation of Q and K head dims.
- Scores are computed transposed (scoresT[k, q]) so the softmax
  denominator falls out of the attn@V matmul via an appended ones column
  on V, and no PE transposes of attention weights are needed.
"""

import sys

for _p in ("/opt/trn_rl_repo",):
    if _p not in sys.path:
        sys.path.insert(0, _p)

from contextlib import ExitStack

import numpy as np

import concourse.bass as bass
import concourse.mybir as mybir
from concourse import bacc
from concourse.masks import make_identity
from concourse.tile import TileContext

B, S, D = 2, 2048, 1024
H, DK = 16, 64
NCORES = 8
CPB = NCORES // B  # cores per batch = 4
HPC = H // CPB  # heads per core = 4
HD = HPC * DK  # 256 output dims per core per projection
THETA = 10000.0

ST = 128  # sequence tile
NST = S // ST  # 16
KTD = 128  # contraction tile over model dim
NKT = D // KTD  # 8
QC = 512  # query chunk in attention
NQC = S // QC  # 4
QTPC = QC // ST  # 4 query tiles per chunk

F32 = mybir.dt.float32
F32R = mybir.dt.float32r
BF16 = mybir.dt.bfloat16


def build_nc():
    nc = bacc.Bacc(
        "TRN2", target_bir_lowering=False, debug=False, num_devices=NCORES
    )
    xT = nc.dram_tensor("xT", [D, S], BF16, kind="ExternalInput").ap()
    wqT = nc.dram_tensor("wqT", [D, HD], BF16, kind="ExternalInput").ap()
    wkT = nc.dram_tensor("wkT", [D, HD], BF16, kind="ExternalInput").ap()
    wvT = nc.dram_tensor("wvT", [D, HD], BF16, kind="ExternalInput").ap()
    woT = nc.dram_tensor("woT", [HD, D], BF16, kind="ExternalInput").ap()
    c1 = nc.dram_tensor("c1", [S, HD], F32, kind="ExternalInput").ap()
    c2 = nc.dram_tensor("c2", [S, HD], F32, kind="ExternalInput").ap()
    part = nc.dram_tensor("part", [S, D], F32, kind="ExternalOutput").ap()

    with TileContext(nc) as tc:
        _body(tc, xT, wqT, wkT, wvT, woT, c1, c2, part)
    nc.compile()
    return nc


def _body(tc, xT, wqT, wkT, wvT, woT, c1, c2, part):
    nc = tc.nc
    with ExitStack() as ctx:
        consts = ctx.enter_context(tc.tile_pool(name="consts", bufs=1))

        # Resident SBUF tensors
        wq_sb = consts.tile([128, NKT, HD], BF16)
        wk_sb = consts.tile([128, NKT, HD], BF16)
        wv_sb = consts.tile([128, NKT, HD], BF16)
        wo_sb = consts.tile([128, HD // 128, D], BF16)
        c1_sb = consts.tile([128, NST, HD], F32)
        c2_sb = consts.tile([128, NST, HD], F32)
        ident = consts.tile([128, 128], F32)
        v65 = consts.tile([128, NST, HPC * 65], BF16)
        # Transposed rope'd Q/K, bf16: a = heads 0,1  b = heads 2,3
        qta = consts.tile([128, NST, ST], BF16)
        qtb = consts.tile([128, NST, ST], BF16)
        kta = consts.tile([128, NST, ST], BF16)
        ktb = consts.tile([128, NST, ST], BF16)
        # Attention outputs (normalized), natural layout, fp32
        anat = consts.tile([128, NST, HPC, DK], F32)
        # Transposed attention outputs for the output projection
        outta = consts.tile([128, NST, ST], BF16)
        outtb = consts.tile([128, NST, ST], BF16)

        nc.sync.dma_start(wq_sb[:], wqT.rearrange("(kt p) h -> p kt h", p=128))
        nc.sync.dma_start(wk_sb[:], wkT.rearrange("(kt p) h -> p kt h", p=128))
        nc.sync.dma_start(wv_sb[:], wvT.rearrange("(kt p) h -> p kt h", p=128))
        nc.sync.dma_start(wo_sb[:], woT.rearrange("(i p) o -> p i o", p=128))
        nc.sync.dma_start(c1_sb[:], c1.rearrange("(st p) h -> p st h", p=128))
        nc.sync.dma_start(c2_sb[:], c2.rearrange("(st p) h -> p st h", p=128))
        make_identity(nc, ident[:])
        nc.vector.memset(v65[:], 1.0)

        qta_f = qta.rearrange("p a b -> p (a b)")
        qtb_f = qtb.rearrange("p a b -> p (a b)")
        kta_f = kta.rearrange("p a b -> p (a b)")
        ktb_f = ktb.rearrange("p a b -> p (a b)")

        # ---- Stage 1: QKV projections + RoPE + transposes ----
        with (
            tc.tile_pool(name="s1x", bufs=3) as s1x,
            tc.tile_pool(name="s1ps", bufs=4, space="PSUM") as s1ps,
            tc.tile_pool(name="s1tmp", bufs=3) as s1tmp,
            tc.tile_pool(name="s1nat", bufs=3) as s1nat,
        ):
            for st in range(NST):
                xt = s1x.tile([128, NKT, ST], BF16)
                nc.sync.dma_start(
                    xt[:],
                    xT[:, st * ST : (st + 1) * ST].rearrange(
                        "(kt p) s -> p kt s", p=128
                    ),
                )

                nat = {}
                for name, w_sb in (("q", wq_sb), ("k", wk_sb), ("v", wv_sb)):
                    ps = s1ps.tile([128, HD], F32, tag="qkvps")
                    for kt in range(NKT):
                        nc.tensor.matmul(
                            ps[:],
                            lhsT=xt[:, kt, :],
                            rhs=w_sb[:, kt, :],
                            start=(kt == 0),
                            stop=(kt == NKT - 1),
                        )
                    if name == "v":
                        # scatter the 4 heads into the 65-wide per-head slots
                        nc.scalar.copy(
                            out=v65[:, st, :].rearrange("p (h e) -> p h e", h=HPC)[
                                :, :, 0:DK
                            ],
                            in_=ps[:].rearrange("p (h e) -> p h e", h=HPC),
                        )
                    else:
                        t1 = s1tmp.tile([128, HD], F32, tag="t1")
                        t2 = s1tmp.tile([128, HD], F32, tag="t2")
                        nc.vector.tensor_mul(t1[:], ps[:], c1_sb[:, st, :])
                        nc.vector.tensor_mul(t2[:], ps[:], c2_sb[:, st, :])
                        nt = s1nat.tile([128, HD], BF16, tag=f"{name}nat")
                        ntv = nt.rearrange("p (h two e) -> p h two e", h=HPC, two=2)
                        t1v = t1.rearrange("p (h two e) -> p h two e", h=HPC, two=2)
                        t2v = t2.rearrange("p (h two e) -> p h two e", h=HPC, two=2)
                        # even outputs: qe*cos - qo*sin
                        nc.vector.tensor_sub(
                            ntv[:, :, 0, :], t1v[:, :, 0, :], t1v[:, :, 1, :]
                        )
                        # odd outputs: qe*sin + qo*cos
                        nc.vector.tensor_add(
                            ntv[:, :, 1, :], t2v[:, :, 0, :], t2v[:, :, 1, :]
                        )
                        nat[name] = nt

                nc.sync.dma_start_transpose(
                    qta[:, st, :], nat["q"][:, 0:128]
                )
                nc.sync.dma_start_transpose(
                    qtb[:, st, :], nat["q"][:, 128:256]
                )
                nc.sync.dma_start_transpose(
                    kta[:, st, :], nat["k"][:, 0:128]
                )
                nc.sync.dma_start_transpose(
                    ktb[:, st, :], nat["k"][:, 128:256]
                )

        # ---- Stage 2: causal attention ----
        with (
            tc.tile_pool(name="s2ps", bufs=2, space="PSUM") as s2ps,
            tc.tile_pool(name="s2pa", bufs=4, space="PSUM") as s2pa,
            tc.tile_pool(name="s2exp", bufs=3) as s2exp,
            tc.tile_pool(name="s2r", bufs=4) as s2r,
        ):
            for qc in range(NQC):
                pattn = []
                for qtl in range(QTPC):
                    pa = s2pa.tile([128, HPC * 65], F32, tag="pattn")
                    pattn.append(pa)
                for kt in range(4 * qc + 4):
                    # local column where valid (q >= k) region starts
                    c0 = max(0, kt * ST - qc * QC)
                    nv = QC - c0
                    for hp, (qt_, kt_) in enumerate(
                        ((qta_f, kta_f), (qtb_f, ktb_f))
                    ):
                        pst = s2ps.tile([128, 2, QC], F32, tag="pst")
                        for hl in range(2):
                            po = 64 * hl
                            nc.tensor.matmul(
                                pst[:, hl, c0:QC],
                                lhsT=kt_[po : po + 64, kt * ST : (kt + 1) * ST],
                                rhs=qt_[po : po + 64, qc * QC + c0 : (qc + 1) * QC],
                                start=True,
                                stop=True,
                            )
                        et = s2exp.tile([128, 2, QC], BF16, tag="exp")
                        nc.scalar.activation(
                            out=et[:, :, c0:QC],
                            in_=pst[:, :, c0:QC],
                            func=mybir.ActivationFunctionType.Exp,
                            scale=1.0 / (DK**0.5),
                        )
                        if c0 > 0 or kt * ST == qc * QC:
                            # diagonal block: zero the strictly-lower part
                            for hl in range(2):
                                nc.gpsimd.affine_select(
                                    out=et[:, hl, c0 : c0 + ST],
                                    in_=et[:, hl, c0 : c0 + ST],
                                    compare_op=mybir.AluOpType.is_ge,
                                    fill=0.0,
                                    base=0,
                                    pattern=[[1, ST]],
                                    channel_multiplier=-1,
                                )
                        for qtl in range(QTPC):
                            qt = qc * QTPC + qtl
                            if qt < kt:
                                continue
                            for hl in range(2):
                                h = hp * 2 + hl
                                # one accumulation group per pattn bank:
                                # start zeroes the whole bank lazily
                                nc.tensor.matmul(
                                    pattn[qtl][:, h * 65 : h * 65 + 65],
                                    lhsT=et[:, hl, qtl * ST : (qtl + 1) * ST],
                                    rhs=v65[:, kt, h * 65 : h * 65 + 65],
                                    start=(kt == 0 and h == 0),
                                    stop=(kt == qt and h == HPC - 1),
                                )
                for qtl in range(QTPC):
                    qt = qc * QTPC + qtl
                    pa = pattn[qtl]
                    rt = s2r.tile([128, HPC], F32, tag="recip")
                    nc.vector.reciprocal(
                        rt[:], bass.AP(pa.tensor, pa.offset + 64, [pa.ap[0], [65, HPC]])
                    )
                    for h in range(HPC):
                        nc.scalar.mul(
                            out=anat[:, qt, h, :],
                            in_=pa[:, h * 65 : h * 65 + 64],
                            mul=rt[:, h : h + 1],
                        )

        # ---- Stage 3: transpose attention outputs + output projection ----
        with (
            tc.tile_pool(name="s3pt", bufs=2, space="PSUM") as s3pt,
            tc.tile_pool(name="s3po", bufs=2, space="PSUM") as s3po,
            tc.tile_pool(name="s3o", bufs=3) as s3o,
        ):
            for st in range(NST):
                for hp, outt in ((0, outta), (1, outtb)):
                    pt = s3pt.tile([128, ST], F32, tag="ptr")
                    # transpose the [s=128, 2*DK=128] block of 2 heads at once
                    nc.tensor.transpose(
                        pt[:],
                        anat[:, st, hp * 2 : hp * 2 + 2, :],
                        ident[:],
                    )
                    nc.vector.tensor_copy(outt[:, st, :], pt[:])

            outta_f = outta.rearrange("p a b -> p (a b)")
            outtb_f = outtb.rearrange("p a b -> p (a b)")
            for st in range(NST):
                for oc in range(2):
                    po = s3po.tile([128, 512], F32, tag="pout")
                    for i, of in enumerate((outta_f, outtb_f)):
                        nc.tensor.matmul(
                            po[:],
                            lhsT=of[:, st * ST : (st + 1) * ST],
                            rhs=wo_sb[:, i, oc * 512 : (oc + 1) * 512],
                            start=(i == 0),
                            stop=(i == 1),
                        )
                    og = s3o.tile([128, 512], F32, tag="ostg")
                    if oc == 0:
                        nc.vector.tensor_copy(og[:], po[:])
                    else:
                        nc.scalar.copy(og[:], po[:])
                    nc.sync.dma_start(
                        part[st * ST : (st + 1) * ST, oc * 512 : (oc + 1) * 512],
                        og[:],
                    )


_NC_CACHE = None


def _get_nc():
    global _NC_CACHE
    if _NC_CACHE is None:
        _NC_CACHE = build_nc()
    return _NC_CACHE


def prep_in_maps(x, token_positions, Wq, Wk, Wv, Wo):
    x = np.asarray(x, dtype=np.float32)
    pos = np.asarray(token_positions)
    Wq = np.asarray(Wq, dtype=np.float32)
    Wk = np.asarray(Wk, dtype=np.float32)
    Wv = np.asarray(Wv, dtype=np.float32)
    Wo = np.asarray(Wo, dtype=np.float32)

    # deinterleave permutation within each head: even dims then odd dims
    deint = np.concatenate([np.arange(0, DK, 2), np.arange(1, DK, 2)])

    inv_freq = (THETA ** (-(np.arange(0, DK, 2, dtype=np.float32) / DK))).astype(
        np.float32
    )

    import ml_dtypes

    bf16 = ml_dtypes.bfloat16
    xT_b = [np.ascontiguousarray(x[b].T).astype(bf16) for b in range(B)]
    c1_b, c2_b = [], []
    for b in range(B):
        ang = pos[b].astype(np.float32)[:, None] * inv_freq[None, :]
        cos = np.cos(ang).astype(np.float32)
        sin = np.sin(ang).astype(np.float32)
        c1 = np.concatenate([cos, sin], axis=1)  # [S, 64]
        c2 = np.concatenate([sin, cos], axis=1)
        c1_b.append(np.ascontiguousarray(np.tile(c1, (1, HPC))))
        c2_b.append(np.ascontiguousarray(np.tile(c2, (1, HPC))))

    in_maps = []
    for c in range(NCORES):
        b = c // CPB
        h0 = (c % CPB) * HPC
        cols_p = np.concatenate([(h0 + i) * DK + deint for i in range(HPC)])
        cols_n = np.arange(h0 * DK, (h0 + HPC) * DK)
        in_maps.append(
            {
                "xT": xT_b[b],
                "wqT": np.ascontiguousarray(Wq.T[:, cols_p]).astype(bf16),
                "wkT": np.ascontiguousarray(Wk.T[:, cols_p]).astype(bf16),
                "wvT": np.ascontiguousarray(Wv.T[:, cols_n]).astype(bf16),
                "woT": np.ascontiguousarray(Wo.T[cols_n, :]).astype(bf16),
                "c1": c1_b[b],
                "c2": c2_b[b],
            }
        )
    return in_maps


def gather(results):
    out = np.zeros((B, S, D), dtype=np.float32)
    for c, res in enumerate(results.results if hasattr(results, "results") else results):
        out[c // CPB] += res["part"]
    return out


def kernel(x, token_positions, Wq, Wk, Wv, Wo, _trace=False, _results_box=None):
    from concourse.bass_utils import run_bass_kernel_spmd

    nc = _get_nc()
    in_maps = prep_in_maps(x, token_positions, Wq, Wk, Wv, Wo)
    res = run_bass_kernel_spmd(
        nc, in_maps, core_ids=list(range(NCORES)), trace=_trace
    )
    if _results_box is not None:
        _results_box.append(res)
    return gather(res.results)


# revision 15
# speedup vs baseline: 1.4557x; 1.4557x over previous
"""Trainium2 Bass kernel: multi-head self-attention with RoPE (causal).

Sharding: 8 cores = 2 batches x 4 head-groups. Core c handles batch c//4
and heads [4*(c%4), 4*(c%4)+4). Each core computes Q/K/V projections for
its 4 heads, RoPE, causal flash-style attention, and a partial output
projection (its heads' contribution to out @ Wo.T). Host sums the 4
partials per batch.

Layout notes:
- Weights are fed pre-transposed ([in, out]) so projections run with the
  contraction dim on partitions.
- Wq/Wk columns are permuted per head (even dims then odd dims) so RoPE
  becomes two 32-wide column-block ops instead of stride-2 ops. Scores
  are invariant to a shared permutation of Q and K head dims.
- Scores are computed transposed (scoresT[k, q]) so the softmax
  denominator falls out of the attn@V matmul via an appended ones column
  on V, and no PE transposes of attention weights are needed.
"""

import sys

for _p in ("/opt/trn_rl_repo",):
    if _p not in sys.path:
        sys.path.insert(0, _p)

from contextlib import ExitStack

import numpy as np

import concourse.bass as bass
import concourse.mybir as mybir
from concourse import bacc
from concourse.masks import make_identity
from concourse.tile import TileContext

B, S, D = 2, 2048, 1024
H, DK = 16, 64
NCORES = 8
CPB = NCORES // B  # cores per batch = 4
HPC = H // CPB  # heads per core = 4
HD = HPC * DK  # 256 output dims per core per projection
THETA = 10000.0

ST = 128  # sequence tile
NST = S // ST  # 16
KTD = 128  # contraction tile over model dim
NKT = D // KTD  # 8
QC = 512  # query chunk in attention
NQC = S // QC  # 4
QTPC = QC // ST  # 4 query tiles per chunk

F32 = mybir.dt.float32
F32R = mybir.dt.float32r
BF16 = mybir.dt.bfloat16


def build_nc():
    nc = bacc.Bacc(
        "TRN2", target_bir_lowering=False, debug=False, num_devices=NCORES
    )
    xT = nc.dram_tensor("xT", [D, S], BF16, kind="ExternalInput").ap()
    wqT = nc.dram_tensor("wqT", [D, HD], BF16, kind="ExternalInput").ap()
    wkT = nc.dram_tensor("wkT", [D, HD], BF16, kind="ExternalInput").ap()
    wvT = nc.dram_tensor("wvT", [D, HD], BF16, kind="ExternalInput").ap()
    woT = nc.dram_tensor("woT", [HD, D], BF16, kind="ExternalInput").ap()
    c1 = nc.dram_tensor("c1", [S, HD], F32, kind="ExternalInput").ap()
    c2 = nc.dram_tensor("c2", [S, HD], F32, kind="ExternalInput").ap()
    part = nc.dram_tensor("part", [S, D], F32, kind="ExternalOutput").ap()

    with TileContext(nc) as tc:
        _body(tc, xT, wqT, wkT, wvT, woT, c1, c2, part)
    nc.compile()
    return nc


def _body(tc, xT, wqT, wkT, wvT, woT, c1, c2, part):
    nc = tc.nc
    with ExitStack() as ctx:
        consts = ctx.enter_context(tc.tile_pool(name="consts", bufs=1))

        # Resident SBUF tensors
        wq_sb = consts.tile([128, NKT, HD], BF16)
        wk_sb = consts.tile([128, NKT, HD], BF16)
        wv_sb = consts.tile([128, NKT, HD], BF16)
        wo_sb = consts.tile([128, HD // 128, D], BF16)
        c1_sb = consts.tile([128, NST, HD], F32)
        c2_sb = consts.tile([128, NST, HD], F32)
        ident = consts.tile([128, 128], F32)
        identb = consts.tile([128, 128], BF16)
        v65 = consts.tile([128, NST, HPC * 65], BF16)
        # Transposed rope'd Q/K, bf16: a = heads 0,1  b = heads 2,3
        qta = consts.tile([128, NST, ST], BF16)
        qtb = consts.tile([128, NST, ST], BF16)
        kta = consts.tile([128, NST, ST], BF16)
        ktb = consts.tile([128, NST, ST], BF16)
        # Attention outputs (normalized), natural layout, fp32
        anat = consts.tile([128, NST, HPC, DK], F32)
        # Transposed attention outputs for the output projection
        outta = consts.tile([128, NST, ST], BF16)
        outtb = consts.tile([128, NST, ST], BF16)

        nc.sync.dma_start(wq_sb[:], wqT.rearrange("(kt p) h -> p kt h", p=128))
        nc.sync.dma_start(wk_sb[:], wkT.rearrange("(kt p) h -> p kt h", p=128))
        nc.sync.dma_start(wv_sb[:], wvT.rearrange("(kt p) h -> p kt h", p=128))
        nc.sync.dma_start(wo_sb[:], woT.rearrange("(i p) o -> p i o", p=128))
        nc.sync.dma_start(c1_sb[:], c1.rearrange("(st p) h -> p st h", p=128))
        nc.sync.dma_start(c2_sb[:], c2.rearrange("(st p) h -> p st h", p=128))
        make_identity(nc, ident[:])
        make_identity(nc, identb[:])
        nc.vector.memset(v65[:], 1.0)

        qta_f = qta.rearrange("p a b -> p (a b)")
        qtb_f = qtb.rearrange("p a b -> p (a b)")
        kta_f = kta.rearrange("p a b -> p (a b)")
        ktb_f = ktb.rearrange("p a b -> p (a b)")

        # ---- Stage 1: QKV projections + RoPE + transposes ----
        XB = 4  # s-tiles per x DMA batch
        with (
            tc.tile_pool(name="s1x", bufs=2) as s1x,
            tc.tile_pool(name="s1ps", bufs=4, space="PSUM") as s1ps,
            tc.tile_pool(name="s1pt", bufs=4, space="PSUM") as s1pt,
            tc.tile_pool(name="s1tmp", bufs=3) as s1tmp,
            tc.tile_pool(name="s1nat", bufs=3) as s1nat,
        ):
            for stb in range(NST // XB):
                xt = s1x.tile([128, NKT, XB * ST], BF16)
                nc.sync.dma_start(
                    xt[:],
                    xT[:, stb * XB * ST : (stb + 1) * XB * ST].rearrange(
                        "(kt p) s -> p kt s", p=128
                    ),
                )
                for stl in range(XB):
                    st = stb * XB + stl
                    nat = {}
                    for name, w_sb in (("q", wq_sb), ("k", wk_sb), ("v", wv_sb)):
                        ps = s1ps.tile([128, HD], F32, tag="qkvps")
                        for kt in range(NKT):
                            nc.tensor.matmul(
                                ps[:],
                                lhsT=xt[:, kt, stl * ST : (stl + 1) * ST],
                                rhs=w_sb[:, kt, :],
                                start=(kt == 0),
                                stop=(kt == NKT - 1),
                            )
                        if name == "v":
                            # scatter 4 heads into the 65-wide per-head slots
                            nc.scalar.copy(
                                out=v65[:, st, :].rearrange(
                                    "p (h e) -> p h e", h=HPC
                                )[:, :, 0:DK],
                                in_=ps[:].rearrange("p (h e) -> p h e", h=HPC),
                            )
                        else:
                            t1 = s1tmp.tile([128, HD], F32, tag="t1")
                            t2 = s1tmp.tile([128, HD], F32, tag="t2")
                            nc.vector.tensor_mul(t1[:], ps[:], c1_sb[:, st, :])
                            nc.vector.tensor_mul(t2[:], ps[:], c2_sb[:, st, :])
                            nt = s1nat.tile([128, HD], BF16, tag=f"{name}nat")
                            ntv = nt.rearrange(
                                "p (h two e) -> p h two e", h=HPC, two=2
                            )
                            t1v = t1.rearrange(
                                "p (h two e) -> p h two e", h=HPC, two=2
                            )
                            t2v = t2.rearrange(
                                "p (h two e) -> p h two e", h=HPC, two=2
                            )
                            # even outputs: qe*cos - qo*sin
                            nc.vector.tensor_sub(
                                ntv[:, :, 0, :], t1v[:, :, 0, :], t1v[:, :, 1, :]
                            )
                            # odd outputs: qe*sin + qo*cos
                            nc.vector.tensor_add(
                                ntv[:, :, 1, :], t2v[:, :, 0, :], t2v[:, :, 1, :]
                            )
                            nat[name] = nt

                    # PE transposes to [dk, s] layout; evac split DVE/ACT
                    for name, dsts in (("q", (qta, qtb)), ("k", (kta, ktb))):
                        for half, dst in enumerate(dsts):
                            pt = s1pt.tile([128, ST], BF16, tag="ptq")
                            nc.tensor.transpose(
                                pt[:],
                                nat[name][:, half * 128 : (half + 1) * 128],
                                identb[:],
                            )
                            if name == "q":
                                nc.vector.tensor_copy(dst[:, st, :], pt[:])
                            else:
                                nc.scalar.copy(dst[:, st, :], pt[:])

        # ---- Stage 2: causal attention ----
        with (
            tc.tile_pool(name="s2ps", bufs=2, space="PSUM") as s2ps,
            tc.tile_pool(name="s2pa", bufs=4, space="PSUM") as s2pa,
            tc.tile_pool(name="s2exp", bufs=3) as s2exp,
            tc.tile_pool(name="s2r", bufs=4) as s2r,
        ):
            for qc in range(NQC):
                pattn = []
                for qtl in range(QTPC):
                    pa = s2pa.tile([128, HPC * 65], F32, tag="pattn")
                    pattn.append(pa)
                for kt in range(4 * qc + 4):
                    # local column where valid (q >= k) region starts
                    c0 = max(0, kt * ST - qc * QC)
                    nv = QC - c0
                    for hp, (qt_, kt_) in enumerate(
                        ((qta_f, kta_f), (qtb_f, ktb_f))
                    ):
                        pst = s2ps.tile([128, 2, QC], F32, tag="pst")
                        for hl in range(2):
                            po = 64 * hl
                            nc.tensor.matmul(
                                pst[:, hl, c0:QC],
                                lhsT=kt_[po : po + 64, kt * ST : (kt + 1) * ST],
                                rhs=qt_[po : po + 64, qc * QC + c0 : (qc + 1) * QC],
                                start=True,
                                stop=True,
                            )
                        et = s2exp.tile([128, 2, QC], BF16, tag="exp")
                        nc.scalar.activation(
                            out=et[:, :, c0:QC],
                            in_=pst[:, :, c0:QC],
                            func=mybir.ActivationFunctionType.Exp,
                            scale=1.0 / (DK**0.5),
                        )
                        if c0 > 0 or kt * ST == qc * QC:
                            # diagonal block: zero the strictly-lower part
                            for hl in range(2):
                                nc.gpsimd.affine_select(
                                    out=et[:, hl, c0 : c0 + ST],
                                    in_=et[:, hl, c0 : c0 + ST],
                                    compare_op=mybir.AluOpType.is_ge,
                                    fill=0.0,
                                    base=0,
                                    pattern=[[1, ST]],
                                    channel_multiplier=-1,
                                )
                        for qtl in range(QTPC):
                            qt = qc * QTPC + qtl
                            if qt < kt:
                                continue
                            for hl in range(2):
                                h = hp * 2 + hl
                                # one accumulation group per pattn bank:
                                # start zeroes the whole bank lazily
                                nc.tensor.matmul(
                                    pattn[qtl][:, h * 65 : h * 65 + 65],
                                    lhsT=et[:, hl, qtl * ST : (qtl + 1) * ST],
                                    rhs=v65[:, kt, h * 65 : h * 65 + 65],
                                    start=(kt == 0 and h == 0),
                                    stop=(kt == qt and h == HPC - 1),
                                )
                for qtl in range(QTPC):
                    qt = qc * QTPC + qtl
                    pa = pattn[qtl]
                    rt = s2r.tile([128, HPC], F32, tag="recip")
                    nc.vector.reciprocal(
                        rt[:], bass.AP(pa.tensor, pa.offset + 64, [pa.ap[0], [65, HPC]])
                    )
                    for h in range(HPC):
                        nc.vector.tensor_scalar_mul(
                            anat[:, qt, h, :],
                            pa[:, h * 65 : h * 65 + 64],
                            rt[:, h : h + 1],
                        )

        # ---- Stage 3: transpose attention outputs + output projection ----
        with (
            tc.tile_pool(name="s3pt", bufs=2, space="PSUM") as s3pt,
            tc.tile_pool(name="s3po", bufs=2, space="PSUM") as s3po,
            tc.tile_pool(name="s3o", bufs=3) as s3o,
        ):
            for st in range(NST):
                for hp, outt in ((0, outta), (1, outtb)):
                    pt = s3pt.tile([128, ST], F32, tag="ptr")
                    # transpose the [s=128, 2*DK=128] block of 2 heads at once
                    nc.tensor.transpose(
                        pt[:],
                        anat[:, st, hp * 2 : hp * 2 + 2, :],
                        ident[:],
                    )
                    nc.vector.tensor_copy(outt[:, st, :], pt[:])

            outta_f = outta.rearrange("p a b -> p (a b)")
            outtb_f = outtb.rearrange("p a b -> p (a b)")
            for st in range(NST):
                og = s3o.tile([128, 2, 512], F32, tag="ostg")
                for oc in range(2):
                    po = s3po.tile([128, 512], F32, tag="pout")
                    for i, of in enumerate((outta_f, outtb_f)):
                        nc.tensor.matmul(
                            po[:],
                            lhsT=of[:, st * ST : (st + 1) * ST],
                            rhs=wo_sb[:, i, oc * 512 : (oc + 1) * 512],
                            start=(i == 0),
                            stop=(i == 1),
                        )
                    if oc == 0:
                        nc.vector.tensor_copy(og[:, oc, :], po[:])
                    else:
                        nc.scalar.copy(og[:, oc, :], po[:])
                nc.sync.dma_start(
                    part[st * ST : (st + 1) * ST, :],
                    og.rearrange("p a b -> p (a b)"),
                )


_NC_CACHE = None


def _get_nc():
    global _NC_CACHE
    if _NC_CACHE is None:
        _NC_CACHE = build_nc()
    return _NC_CACHE


def prep_in_maps(x, token_positions, Wq, Wk, Wv, Wo):
    x = np.asarray(x, dtype=np.float32)
    pos = np.asarray(token_positions)
    Wq = np.asarray(Wq, dtype=np.float32)
    Wk = np.asarray(Wk, dtype=np.float32)
    Wv = np.asarray(Wv, dtype=np.float32)
    Wo = np.asarray(Wo, dtype=np.float32)

    # deinterleave permutation within each head: even dims then odd dims
    deint = np.concatenate([np.arange(0, DK, 2), np.arange(1, DK, 2)])

    inv_freq = (THETA ** (-(np.arange(0, DK, 2, dtype=np.float32) / DK))).astype(
        np.float32
    )

    import ml_dtypes

    bf16 = ml_dtypes.bfloat16
    xT_b = [np.ascontiguousarray(x[b].T).astype(bf16) for b in range(B)]
    c1_b, c2_b = [], []
    for b in range(B):
        ang = pos[b].astype(np.float32)[:, None] * inv_freq[None, :]
        cos = np.cos(ang).astype(np.float32)
        sin = np.sin(ang).astype(np.float32)
        c1 = np.concatenate([cos, sin], axis=1)  # [S, 64]
        c2 = np.concatenate([sin, cos], axis=1)
        c1_b.append(np.ascontiguousarray(np.tile(c1, (1, HPC))))
        c2_b.append(np.ascontiguousarray(np.tile(c2, (1, HPC))))

    in_maps = []
    for c in range(NCORES):
        b = c // CPB
        h0 = (c % CPB) * HPC
        cols_p = np.concatenate([(h0 + i) * DK + deint for i in range(HPC)])
        cols_n = np.arange(h0 * DK, (h0 + HPC) * DK)
        in_maps.append(
            {
                "xT": xT_b[b],
                "wqT": np.ascontiguousarray(Wq.T[:, cols_p]).astype(bf16),
                "wkT": np.ascontiguousarray(Wk.T[:, cols_p]).astype(bf16),
                "wvT": np.ascontiguousarray(Wv.T[:, cols_n]).astype(bf16),
                "woT": np.ascontiguousarray(Wo.T[cols_n, :]).astype(bf16),
                "c1": c1_b[b],
                "c2": c2_b[b],
            }
        )
    return in_maps


def gather(results):
    out = np.zeros((B, S, D), dtype=np.float32)
    for c, res in enumerate(results.results if hasattr(results, "results") else results):
        out[c // CPB] += res["part"]
    return out


def kernel(x, token_positions, Wq, Wk, Wv, Wo, _trace=False, _results_box=None):
    from concourse.bass_utils import run_bass_kernel_spmd

    nc = _get_nc()
    in_maps = prep_in_maps(x, token_positions, Wq, Wk, Wv, Wo)
    res = run_bass_kernel_spmd(
        nc, in_maps, core_ids=list(range(NCORES)), trace=_trace
    )
    if _results_box is not None:
        _results_box.append(res)
    return gather(res.results)
